# revision 7
# baseline (speedup 1.0000x reference)
"""GCN layer (2x segment-sum aggregate + linear) on 8 Trainium2 NeuronCores.

Sharding: nodes (and their incident edges, by dst) are partitioned across the
8 cores; the feature table is replicated (in bf16) in each core's HBM.

Per-core algorithm, per aggregation round (all heavy data in bf16, psum f32):
  - edges are grouped on the host into 128-edge "tiles"; each tile's dsts all
    fall in one 128-node block of the (permuted) local node space
  - a hardware For_i loop runs over the 49 blocks; per block the body
    stages that block's gather indices + local-dst columns (dynamic ds()
    slices), builds one-hot S tiles on DVE, dma_gathers the block's src rows
    (lo/hi index streams around the int16 32768 limit), and accumulates
    14 matmuls into PSUM
  - round 1 produces psum[r, feat] = S^T E directly (no transpose), written
    per block to the local h1 slab; an AllGather (bf16) replicates the full
    h1 table; round 2 produces psum[feat, r] = E^T S and feeds the 128x128
    linear (+bias), written per block to the f32 output

The local node ids are permuted on the host (balanced bin packing) so that
every 128-node block needs exactly 9 low + 5 high tiles on every core: the
compiled program is identical across cores (SPMD), only tensors differ.
kernel() un-permutes when assembling the full output.
"""

import numpy as np
import ml_dtypes

import concourse.bass as bass
import concourse.bacc as bacc
import concourse.mybir as mybir
import concourse.tile as tile
from concourse.bass import ds
from concourse.bass_utils import run_bass_kernel_spmd

# ---- problem constants (hardcoded per contest contract) ----
N_NODES = 50000
D = 128
NCORES = 8
PER = N_NODES // NCORES          # 6250 real nodes per core
HSPLIT = 32768                   # int16 index limit split point
NBIN = 49                        # 128-node blocks per core
NSLAB = NBIN * 128               # 6272 padded local node slots
NFULL = NCORES * NSLAB           # 50176 rows in the gathered h1 table
T_LO = 9                         # low-stream tiles per block
T_HI = 5                         # high-stream tiles per block
TT = T_LO + T_HI                 # 14 tiles per block
# per-block gather segments (tiles): 3x384-idx low, 512+128-idx high
SEGS = [(0, 3, 384), (3, 3, 384), (6, 3, 384), (9, 4, 512), (13, 1, 128)]
IDXC = TT * 128 // 16            # 112 wrapped int16 idx cols per block
NT = NBIN * TT                   # 686 tiles per round
PROBE_MODE = None                # None | "rep_noag" | "rep_agonly"

BF16 = mybir.dt.bfloat16
NP_BF16 = ml_dtypes.bfloat16


def _mid_bcast(ap, k):
    """[128, r] AP -> [128, k, r] with the middle dim broadcast."""
    return bass.AP(ap.tensor, ap.offset, [ap.ap[0], [0, k], ap.ap[1]])


def _build_nc(repeat: int = 1, timing_loop: int | None = None):
    nc = bacc.Bacc(
        "TRN2",
        target_bir_lowering=False,
        debug=False,
        num_devices=NCORES,
        num_swdge_queues=1,
    )
    f32, i16 = mybir.dt.float32, mybir.dt.int16

    feature = nc.dram_tensor("feature", [N_NODES, D], BF16, kind="ExternalInput")
    # wrapped int16 gather ids, rounds 1&2: [128, NBIN*IDXC]
    g1 = nc.dram_tensor("g1", [128, NBIN * IDXC], i16, kind="ExternalInput")
    g2 = nc.dram_tensor("g2", [128, NBIN * IDXC], i16, kind="ExternalInput")
    # negated local dst (within-block) per tile slot, rounds 1&2: [128, NBIN*TT]
    nd1 = nc.dram_tensor("nd1", [128, NBIN * TT], f32, kind="ExternalInput")
    nd2 = nc.dram_tensor("nd2", [128, NBIN * TT], f32, kind="ExternalInput")
    w_in = nc.dram_tensor("w_in", [D, D], BF16, kind="ExternalInput")
    b_in = nc.dram_tensor("b_in", [D, 1], f32, kind="ExternalInput")
    out_t = nc.dram_tensor("out_t", [D, NSLAB], f32, kind="ExternalOutput")

    with tile.TileContext(nc) as tc:
        with (
            tc.tile_pool(name="const", bufs=1) as cpool,
            tc.tile_pool(name="idx", bufs=1) as ipool,
            tc.tile_pool(name="stg", bufs=1) as stpool,
            tc.tile_pool(name="ebuf", bufs=1) as epool,
            tc.tile_pool(name="sel", bufs=1) as spool,
            tc.tile_pool(name="fl", bufs=1) as fpool,
            tc.tile_pool(name="ps", bufs=2, space="PSUM") as pspool,
            tc.tile_pool(name="ps2", bufs=2, space="PSUM") as ps2pool,
            tc.tile_pool(name="dram", bufs=1, space="DRAM") as dpool,
        ):
            g_t = {r: ipool.tile([128, NBIN * IDXC], i16, tag=f"g{r}",
                                 name=f"g{r}_t") for r in (1, 2)}
            nd_t = {r: ipool.tile([128, NBIN * TT], f32, tag=f"nd{r}",
                                  name=f"nd{r}_t") for r in (1, 2)}
            nc.sync.dma_start(out=g_t[1][:], in_=g1[:])
            nc.sync.dma_start(out=g_t[2][:], in_=g2[:])
            nc.sync.dma_start(out=nd_t[1][:], in_=nd1[:])
            nc.sync.dma_start(out=nd_t[2][:], in_=nd2[:])

            w_t = cpool.tile([D, D], BF16)
            b_t = cpool.tile([D, 1], f32)
            nc.sync.dma_start(out=w_t[:], in_=w_in[:])
            nc.sync.dma_start(out=b_t[:], in_=b_in[:])
            # neg_iota[p, r] = -r  (f32 ints <=128: exact)
            neg_iota = cpool.tile([128, 128], f32)
            nc.gpsimd.iota(neg_iota[:], pattern=[[-1, 128]], base=0,
                           channel_multiplier=0,
                           allow_small_or_imprecise_dtypes=True)

            h1part = dpool.tile([NSLAB, D], BF16)
            h1full = dpool.tile([NFULL, D], BF16)

            def block_body(rnd, b, table, table_hi):
                """Per-block body; b is a loop var (RuntimeValue) or int."""
                idx_stg = stpool.tile([128, IDXC], i16, tag="idxstg")
                nd_stg = stpool.tile([128, TT], f32, tag="ndstg")
                nc.vector.tensor_copy(idx_stg[:], g_t[rnd][:, ds(b * IDXC, IDXC)])
                nc.vector.tensor_copy(nd_stg[:], nd_t[rnd][:, ds(b * TT, TT)])

                # grouped one-hot builds: S[edge, t, r] = (dstL[edge,t]==r)
                S = spool.tile([128, TT * 128], BF16, tag="S")
                nc.vector.tensor_tensor(
                    out=S[:].rearrange("p (t r) -> p t r", r=128),
                    in0=nd_stg[:].to_broadcast([128, TT, 128]),
                    in1=_mid_bcast(neg_iota[:], TT),
                    op=mybir.AluOpType.is_equal)

                eb = epool.tile([128, TT * D], BF16, tag="ebuf")
                icol = 0
                for (t0, ntile, nidx) in SEGS:
                    tab = table if t0 < T_LO else table_hi
                    nc.gpsimd.dma_gather(
                        eb[:, t0 * D:(t0 + ntile) * D]
                            .rearrange("p (n d) -> p n d", d=D),
                        tab,
                        idx_stg[:, icol:icol + nidx // 16],
                        num_idxs=nidx, num_idxs_reg=nidx,
                        elem_size=D, elem_step=D, queue_num=0)
                    icol += nidx // 16

                ps = pspool.tile([128, 128], f32, tag=f"agg{rnd}")
                for j in range(TT):
                    if rnd == 1:
                        # psum[r, feat] += S_j^T @ E_j
                        nc.tensor.matmul(
                            ps[:],
                            lhsT=S[:, j * 128:(j + 1) * 128],
                            rhs=eb[:, j * D:(j + 1) * D],
                            start=(j == 0), stop=(j == TT - 1))
                    else:
                        # psum[feat, r] += E_j^T @ S_j
                        nc.tensor.matmul(
                            ps[:],
                            lhsT=eb[:, j * D:(j + 1) * D],
                            rhs=S[:, j * 128:(j + 1) * 128],
                            start=(j == 0), stop=(j == TT - 1))

                if rnd == 1:
                    stage = fpool.tile([128, 128], BF16, tag="h1blk")
                    nc.scalar.copy(stage[:], ps[:])
                    nc.sync.dma_start(
                        out=h1part[ds(b * 128, 128), :], in_=stage[:])
                else:
                    h_sb = fpool.tile([128, 128], BF16, tag="hsb")
                    nc.scalar.copy(h_sb[:], ps[:])
                    o_ps = ps2pool.tile([128, 128], f32, tag="ops")
                    nc.tensor.matmul(o_ps[:], lhsT=w_t[:], rhs=h_sb[:],
                                     start=True, stop=True)
                    ob = fpool.tile([128, 128], f32, tag="ob")
                    nc.vector.tensor_scalar_add(ob[:], o_ps[:], b_t[:, 0:1])
                    nc.sync.dma_start(
                        out=out_t[:, ds(b * 128, 128)], in_=ob[:])

            def one_round(rnd, table, table_hi):
                with tc.For_i(0, NBIN, 1) as b:
                    block_body(rnd, b, table, table_hi)

            def allgather():
                nc.gpsimd.collective_compute(
                    "AllGather", mybir.AluOpType.bypass,
                    replica_groups=[list(range(NCORES))],
                    ins=[h1part.opt()], outs=[h1full.opt()])

            if timing_loop is None:
                if PROBE_MODE == "rep_noag":
                    one_round(1, feature[:], feature[HSPLIT:, :])
                    allgather()
                    for _rep in range(repeat):
                        one_round(1, feature[:], feature[HSPLIT:, :])
                        one_round(2, h1full[:], h1full[HSPLIT:, :])
                elif PROBE_MODE == "rep_agonly":
                    one_round(1, feature[:], feature[HSPLIT:, :])
                    for _rep in range(repeat):
                        allgather()
                    one_round(2, h1full[:], h1full[HSPLIT:, :])
                else:
                    for _rep in range(repeat):
                        one_round(1, feature[:], feature[HSPLIT:, :])
                        allgather()
                        one_round(2, h1full[:], h1full[HSPLIT:, :])
            else:
                one_round(1, feature[:], feature[HSPLIT:, :])
                allgather()
                with tc.For_i(0, timing_loop, 1):
                    block_body(1, 0, feature[:], feature[HSPLIT:, :])
                    block_body(2, 0, h1full[:], h1full[HSPLIT:, :])
    nc.compile()
    return nc


_NC_CACHE: dict = {}


def get_nc(repeat: int = 1):
    if repeat not in _NC_CACHE:
        _NC_CACHE[repeat] = _build_nc(repeat)
    return _NC_CACHE[repeat]


def _wrap_idx(idx: np.ndarray) -> np.ndarray:
    """[n] -> [128, n//16] int16 wrapped layout (16-partition wrap, 8x
    replicated for the Q7 cores)."""
    n = idx.shape[0]
    w = idx.reshape(n // 16, 16).T.astype(np.int16)
    return np.ascontiguousarray(np.tile(w, (8, 1)))


def _pack_bins(deg_lo: np.ndarray, deg: np.ndarray):
    """Balanced snake packing of PER nodes into NBIN bins (<=128 nodes each).
    Returns perm: perm[orig_local] = bin*128 + slot."""
    order = np.argsort(-(deg_lo * 2 + deg), kind="stable")
    bins = [[] for _ in range(NBIN)]
    pos, fwd = 0, True
    for n in order:
        for _ in range(NBIN + 1):
            if len(bins[pos]) < 128:
                break
            pos, fwd = _step(pos, fwd, NBIN)
        bins[pos].append(n)
        pos, fwd = _step(pos, fwd, NBIN)
    perm = np.empty(PER, np.int64)
    for bi, members in enumerate(bins):
        for sl, n in enumerate(members):
            perm[n] = bi * 128 + sl
    return perm


def _step(pos, fwd, nbins):
    if fwd:
        if pos + 1 >= nbins:
            return pos, False
        return pos + 1, True
    if pos - 1 < 0:
        return pos, True
    return pos - 1, False


def _build_round_tensors(g_src: np.ndarray, dslot: np.ndarray):
    """Per-edge gather ids + permuted local dst slots -> (g, nd).

    g: [128, NBIN*IDXC] wrapped int16; per block the idx stream is the
    block's lo edges (tiles 0..8) then hi edges (tiles 9..13), hi ids
    shifted down by HSPLIT.
    nd: [128, NBIN*TT] f32; nd[row, b*TT+t] = -dstL of the edge in tile t
    row `row` of block b; padding slots hold +1 (never matches).
    """
    blk = dslot >> 7
    dstL = dslot & 127
    is_lo = g_src < HSPLIT

    g_all = np.zeros(NBIN * TT * 128, np.int64)
    ndl = np.full((128, NBIN * TT), 1.0, np.float32)
    for b in range(NBIN):
        in_b = blk == b
        base = b * TT * 128
        for stream, t0, t_n in ((0, 0, T_LO), (1, T_LO, T_HI)):
            m = in_b & (is_lo if stream == 0 else ~is_lo)
            gs = g_src[m] - (0 if stream == 0 else HSPLIT)
            dl = dstL[m]
            cnt = gs.shape[0]
            assert cnt <= t_n * 128, (b, stream, cnt, t_n * 128)
            sl = base + t0 * 128
            g_all[sl: sl + cnt] = gs
            col = np.arange(cnt) // 128 + t0 + b * TT
            row = np.arange(cnt) % 128
            ndl[row, col] = -dl.astype(np.float32)
    return _wrap_idx(g_all), ndl


def prep_core_inputs(feature, W, b, src, dst):
    feature = np.ascontiguousarray(np.asarray(feature).astype(NP_BF16))
    W = np.ascontiguousarray(np.asarray(W).astype(NP_BF16))
    b = np.asarray(b, dtype=np.float32).reshape(D, 1)
    src = np.asarray(src).astype(np.int64)
    dst = np.asarray(dst).astype(np.int64)

    owner = dst // PER
    deg_lo_all = np.bincount(dst[src < HSPLIT], minlength=N_NODES)
    deg_all = np.bincount(dst, minlength=N_NODES)
    perms = []
    for c in range(NCORES):
        dlo = deg_lo_all[c * PER:(c + 1) * PER]
        dg = deg_all[c * PER:(c + 1) * PER]
        perms.append(_pack_bins(dlo.astype(np.int64), dg.astype(np.int64)))

    permg = np.empty(N_NODES, np.int64)
    for c in range(NCORES):
        permg[c * PER:(c + 1) * PER] = c * NSLAB + perms[c]

    in_maps = []
    for c in range(NCORES):
        sel = owner == c
        es, ed = src[sel], dst[sel] - c * PER
        dslot = perms[c][ed]
        g1_, nd1_ = _build_round_tensors(es, dslot)
        g2_, nd2_ = _build_round_tensors(permg[es], dslot)
        in_maps.append({
            "feature": feature,
            "g1": g1_, "nd1": nd1_,
            "g2": g2_, "nd2": nd2_,
            "w_in": W, "b_in": b,
        })
    return in_maps, perms


def assemble(results, perms) -> np.ndarray:
    out = np.empty((N_NODES, D), np.float32)
    for c in range(NCORES):
        ot = np.asarray(results[c]["out_t"])       # [D, NSLAB]
        out[c * PER:(c + 1) * PER, :] = ot.T[perms[c], :]
    return out


def kernel(feature, W, b, src, dst) -> np.ndarray:
    nc = get_nc(repeat=1)
    in_maps, perms = prep_core_inputs(feature, W, b, src, dst)
    res = run_bass_kernel_spmd(nc, in_maps, core_ids=list(range(NCORES)))
    return assemble(res.results, perms)
